# revision 2
# baseline (speedup 1.0000x reference)
"""Trainium2 Bass kernel for nn_BERTgridGenerator (segment_reduce).

Strategy (data-parallel over batch: core b handles batch element b):
  - agg[b,0,:]=0 and agg[b,k,:] = (sum of segment (k-1)'s tokens minus its
    first token) / count. With the first token cancelled exactly, only the
    "slot >= 1" member tokens of each segment contribute:
        P[k] = sum_j A_j[k] * (mask[s_kj] / count[k]),   agg[k+1] = P[k]
    where A_j[k] = emb_table[corpus[s_kj]] is fetched on-device with an
    indirect (gather) DMA, and the per-row scalars are host-computed from
    the tiny int index tensors.
  - The grid is a row-gather from agg: since agg[0]==0 identically, the
    reference's where(seg_map>=0, agg[clip(seg_map,0)], 0) == agg[max(seg_map,0)].
    The host paints seg_map (last-write-wins box painting, 170 tiny numpy
    slice assigns), and the device does the heavy part: for each of the 6
    128-row d-chunks, ap_gather (GPSIMD free-dim gather) produces
    grid[d, cell] = aggT[d, seg_map[cell]] as a dense [128, 12288] tile
    that DMAs straight into the [D, H, W]-contiguous output.
"""

import numpy as np

import concourse.bacc as bacc
import concourse.bass as bass
import concourse.mybir as mybir
import concourse.tile as tile
from concourse import bass_utils
from concourse.masks import make_identity

# Problem constants (hardcoded per contract).
B = 8
S = 510
D = 768
K = 170
VOCAB = 30522
STRIDE = 8
HC, WC = 128, 96          # 1024//8, 768//8
NCELL = HC * WC           # 12288
P = 128
ND = D // P               # 6 d-chunks
KC1 = K - P               # 42 rows in k-chunk 1

_prog_cache: dict[int, object] = {}


def _build_program(n_slots: int):
    """Build the SPMD Bass program (same on all 8 cores).

    Inputs (per core):
      emb   [VOCAB, D] f32  - full embedding table (replicated)
      gidx  [P, 2*n_slots] i32 - emb row index for (slot j, k-chunk c) at
                                 column 2*j+c, partition = k within chunk
      sc    [P, 2*n_slots] f32 - per-row scalar mask[s]/count (0 if invalid)
      smap  [P, NCELL//16] i16 - seg_map indices, 16-partition-wrapped and
                                 replicated to all 8 partition groups
    Outputs (per core):
      agg_out  [K, D] f32
      grid_out [D, NCELL] f32  (== [D, HC, WC] contiguous)
    """
    nc = bacc.Bacc("TRN2", target_bir_lowering=False, debug=False, num_devices=B)
    f32 = mybir.dt.float32
    emb = nc.dram_tensor("emb", [VOCAB, D], f32, kind="ExternalInput").ap()
    gidx = nc.dram_tensor("gidx", [P, 2 * n_slots], mybir.dt.int32,
                          kind="ExternalInput").ap()
    sc = nc.dram_tensor("sc", [P, 2 * n_slots], f32, kind="ExternalInput").ap()
    smap = nc.dram_tensor("smap", [P, NCELL // 16], mybir.dt.int16,
                          kind="ExternalInput").ap()
    agg_out = nc.dram_tensor("agg_out", [K, D], f32, kind="ExternalOutput").ap()
    grid_out = nc.dram_tensor("grid_out", [D, NCELL], f32,
                              kind="ExternalOutput").ap()

    with tile.TileContext(nc) as tc:
        with (
            tc.tile_pool(name="small", bufs=1) as sp,
            tc.tile_pool(name="apool", bufs=2) as ap_pool,
            tc.tile_pool(name="gotpool", bufs=2) as gp,
            tc.tile_pool(name="psum", bufs=2, space="PSUM") as pp,
        ):
            idx_t = sp.tile([P, 2 * n_slots], mybir.dt.int32)
            sc_t = sp.tile([P, 2 * n_slots], f32)
            smap_t = sp.tile([P, NCELL // 16], mybir.dt.int16)
            nc.sync.dma_start(out=idx_t[:], in_=gidx[:, :])
            nc.sync.dma_start(out=sc_t[:], in_=sc[:, :])
            nc.sync.dma_start(out=smap_t[:], in_=smap[:, :])

            ident = sp.tile([P, P], f32)
            make_identity(nc, ident[:])

            # P accumulation per k-chunk: p_c[k, :] = sum_j a_j[k, :] * sc_j[k]
            p_chunks = []
            for c in range(2):
                p_c = sp.tile([P, D], f32, tag=f"pchunk{c}")
                for j in range(n_slots):
                    col = 2 * j + c
                    a_t = ap_pool.tile([P, D], f32, tag="aslot")
                    nc.gpsimd.indirect_dma_start(
                        out=a_t[:], out_offset=None, in_=emb[:],
                        in_offset=bass.IndirectOffsetOnAxis(
                            ap=idx_t[:, col:col + 1], axis=0),
                    )
                    if j == 0:
                        nc.vector.tensor_scalar_mul(
                            p_c[:], a_t[:], sc_t[:, col:col + 1])
                    else:
                        nc.vector.tensor_scalar_mul(
                            a_t[:], a_t[:], sc_t[:, col:col + 1])
                        nc.vector.tensor_add(p_c[:], p_c[:], a_t[:])
                p_chunks.append(p_c)
            p0, p1 = p_chunks

            # agg output: row 0 zeros, rows 1..128 = P[0..127],
            # rows 129..169 = P[128..168]
            zrow = sp.tile([1, D], f32)
            nc.vector.memset(zrow[:], 0.0)
            nc.sync.dma_start(out=agg_out[0:1, :], in_=zrow[:])
            nc.sync.dma_start(out=agg_out[1:P + 1, :], in_=p0[:])
            nc.sync.dma_start(out=agg_out[P + 1:K, :], in_=p1[0:KC1 - 1, :])

            # Per d-chunk: transpose P -> aggT[d, k]; gather grid cells; DMA out
            for dci in range(ND):
                ds = dci * P
                ps0 = pp.tile([P, P], f32, tag="ps0")
                nc.tensor.transpose(
                    out=ps0[:], in_=p0[:, ds:ds + P], identity=ident[:])
                ps1 = pp.tile([P, KC1 - 1], f32, tag="ps1")
                nc.tensor.transpose(
                    out=ps1[:], in_=p1[0:KC1 - 1, ds:ds + P],
                    identity=ident[0:KC1 - 1, 0:KC1 - 1])

                agg_t = sp.tile([P, K], f32, tag="aggT")
                nc.vector.memset(agg_t[:, 0:1], 0.0)
                nc.vector.tensor_copy(out=agg_t[:, 1:P + 1], in_=ps0[:])
                nc.vector.tensor_copy(out=agg_t[:, P + 1:K], in_=ps1[:])

                got = gp.tile([P, NCELL], f32, tag="got")
                nc.gpsimd.ap_gather(
                    out_ap=got[:], in_ap=agg_t[:], idxs_ap=smap_t[:],
                    channels=P, num_elems=K, d=1, num_idxs=NCELL,
                )
                nc.sync.dma_start(out=grid_out[ds:ds + P, :], in_=got[:])

    nc.compile()
    return nc


def _prep_batch(corpus_b, mask_b, seg_b, coor_b, n_slots):
    """Host-side index prep for one batch element (all tiny int tensors)."""
    order = np.argsort(seg_b, kind="stable")
    svals = seg_b[order]
    counts = np.bincount(seg_b, minlength=K)[:K].astype(np.int64)
    starts = np.searchsorted(svals, np.arange(K))
    inv = np.float32(1.0) / counts.astype(np.float32)  # inf where count==0

    gidx = np.zeros((P, 2 * n_slots), np.int32)
    sc = np.zeros((P, 2 * n_slots), np.float32)
    for j in range(1, n_slots + 1):
        valid = counts > j
        tok = np.zeros(K, np.int64)
        tok[valid] = order[starts[valid] + j]
        g = corpus_b[tok].astype(np.int32)
        s = mask_b[tok].astype(np.float32) * inv
        g[~valid] = 0
        s[~valid] = 0.0
        col = 2 * (j - 1)
        gidx[:, col] = g[0:P]
        sc[:, col] = s[0:P]
        gidx[0:KC1, col + 1] = g[P:K]
        sc[0:KC1, col + 1] = s[P:K]

    cc = (coor_b // STRIDE).astype(np.int64)
    smap = np.zeros((HC, WC), np.int16)
    for k in range(K):
        x1, y1, x2, y2 = cc[k]
        smap[y1:y2, x1:x2] = k
    wrapped = np.tile(
        np.ascontiguousarray(smap.reshape(NCELL // 16, 16).T), (8, 1)
    ).astype(np.int16)
    return gidx, sc, wrapped


def kernel(emb_table, corpus, mask, seg_indices, coor, image_h, image_w):
    emb_table = np.ascontiguousarray(np.asarray(emb_table, dtype=np.float32))
    corpus = np.asarray(corpus, dtype=np.int32)
    mask = np.asarray(mask, dtype=np.int32)
    seg_indices = np.asarray(seg_indices, dtype=np.int32)
    coor = np.asarray(coor, dtype=np.int32)
    assert int(image_h) // STRIDE == HC and int(image_w) // STRIDE == WC

    max_count = max(
        int(np.bincount(seg_indices[b], minlength=K)[:K].max())
        for b in range(B)
    )
    n_slots = max(max_count - 1, 1)

    if n_slots not in _prog_cache:
        _prog_cache[n_slots] = _build_program(n_slots)
    nc = _prog_cache[n_slots]

    in_maps = []
    for b in range(B):
        gidx, sc, wrapped = _prep_batch(
            corpus[b], mask[b], seg_indices[b], coor[b], n_slots)
        in_maps.append({"emb": emb_table, "gidx": gidx, "sc": sc,
                        "smap": wrapped})

    res = bass_utils.run_bass_kernel_spmd(nc, in_maps, core_ids=list(range(B)),
                                          **_RUN_KWARGS)
    _LAST_RESULT[0] = res
    agg = np.stack([res.results[b]["agg_out"] for b in range(B)])
    grid = np.stack(
        [res.results[b]["grid_out"].reshape(D, HC, WC) for b in range(B)])
    return agg, grid


# test-harness hooks (unused by graders): set _RUN_KWARGS["trace"]=True to
# capture an NTFF profile; the BassKernelResults lands in _LAST_RESULT[0].
_RUN_KWARGS: dict = {}
_LAST_RESULT: list = [None]
